# revision 44
# baseline (speedup 1.0000x reference)
"""Trainium2 Bass kernel for multi-head attention (GQA + RoPE + causal) — v2.

Problem shapes (hardcoded):
  x: (2, 2048, 2048)  Wq: (2048, 2048->512/core)  Wk/Wv: (2048, 512->128/core)
  Wo: (2048->512/core, 2048)  cos/sin: (2048, 64)  mask: causal (1,1,2048,2048)

Sharding: 8 cores = 2 batches (DP) x 4 head groups (TP).  Each core handles
one batch sample and 8 query heads (= 2 KV heads, each kept with its 4 query
heads).  Wo's input dim is sharded, so each core produces a partial
(2048, 2048) output; the host sums the 4 partials per batch (fp32).

v2 changes over the 438us baseline:
  - software pipelining: the QKV projection of seq-block b+1 and the output
    projection of block b-1 are interleaved (generator "filler" units) into
    the exp-bound attention inner loop of block b, so the tensor engine never
    idles waiting on the scalar engine and HAM stays warm.
  - scalar engine runs ONLY the softmax exps; all DMA triggers, psum drains
    and the denominator path moved to vector/gpsimd/sync queues.
  - causal trimming: diagonal k-tiles stream only the valid q-range
    [128*i, 512) through scores/exp/PV, and the triangle masking is a
    128-col affine_select on the vector engine (was full-tile on gpsimd).
  - host pre-tiles x/weights into partition-major [128, c, n] layouts so
    each input needs only 1-4 large DMAs instead of 16 per tensor.
  - the softmax denominator (ones-column PV row 64) is read straight from
    psum by reciprocal_approx_fast, no staging copy.
  - output staged and DMA'd as fp16 (halves write traffic; host sums fp32).
"""

import os
import sys
from collections import deque

import numpy as np

if "/opt/trn_rl_repo" not in sys.path:
    sys.path.insert(0, "/opt/trn_rl_repo")

SEQ = 2048
DIM = 2048
HEAD_DIM = 64
DQ = 512          # query dims per core (8 heads)
DKV = 128         # kv dims per core (2 kv heads)
SCALE = HEAD_DIM ** -0.5
N_CORES = 8
NB = SEQ // 512   # 4 seq blocks of 512
NKT = SEQ // 128  # 16 k-tiles of 128

_PROGRAM_CACHE = {}


class _Filler:
    """FIFO of generator units; attention loops pump a few steps per
    iteration to fill tensor-engine slack while the scalar engine exps."""

    def __init__(self):
        self.q = deque()
        self.done = set()

    def add(self, label, gen_fn):
        self.q.append((label, gen_fn()))

    def step(self, n=1):
        while n > 0 and self.q:
            label, g = self.q[0]
            try:
                next(g)
                n -= 1
            except StopIteration:
                self.done.add(label)
                self.q.popleft()

    def run_until(self, label):
        while label not in self.done and self.q:
            lab, g = self.q[0]
            for _ in g:
                pass
            self.done.add(lab)
            self.q.popleft()

    def run_all(self):
        while self.q:
            lab, g = self.q[0]
            for _ in g:
                pass
            self.done.add(lab)
            self.q.popleft()


def _build_program(causal: bool):
    import concourse.bass as bass  # noqa: F401
    import concourse.mybir as mybir
    from concourse import bacc
    from concourse.masks import make_identity
    from concourse.tile import TileContext

    f32 = mybir.dt.float32
    f16 = mybir.dt.float16
    AOT = mybir.AluOpType
    EXP = mybir.ActivationFunctionType.Exp

    nc = bacc.Bacc(None, target_bir_lowering=False)
    # host-pretiled partition-major layouts
    xT2 = nc.declare_dram_parameter("xT2", [128, 16, SEQ], f16, isOutput=False)
    wq2 = nc.declare_dram_parameter("wq2", [128, 16, DQ], f16, isOutput=False)
    wk2 = nc.declare_dram_parameter("wk2", [128, 16, DKV], f16, isOutput=False)
    wv2 = nc.declare_dram_parameter("wv2", [128, 16, DKV], f16, isOutput=False)
    wo2 = nc.declare_dram_parameter("wo2", [128, 4, DIM], f16, isOutput=False)
    cos2 = nc.declare_dram_parameter("cos2", [128, SEQ], f16, isOutput=False)
    sin2 = nc.declare_dram_parameter("sin2", [128, SEQ], f16, isOutput=False)
    outp = nc.declare_dram_parameter("out", [SEQ, DIM], f16, isOutput=True)

    with TileContext(nc) as tc:
        with tc.tile_pool(name="pa", bufs=1) as pa, \
             tc.tile_pool(name="pw", bufs=1) as pw, \
             tc.tile_pool(name="pp", bufs=1, space="PSUM") as pp:

            # ---------------- persistent SBUF tiles ----------------
            wq_sb = pa.tile([128, 16, DQ], f16, name="wq_sb", tag="wq_sb")
            wk_sb = pa.tile([128, 16, DKV], f16, name="wk_sb", tag="wk_sb")
            wv_sb = pa.tile([128, 16, DKV], f16, name="wv_sb", tag="wv_sb")
            wo_sb = pa.tile([128, 4, DIM], f16, name="wo_sb", tag="wo_sb")
            cos_sb = pa.tile([128, SEQ], f16, name="cos_sb", tag="cos_sb")
            sin_sb = pa.tile([128, SEQ], f16, name="sin_sb", tag="sin_sb")
            identity = pa.tile([128, 128], f16, name="identity", tag="identity")
            qt = [[pa.tile([128, 512], f16, name=f"qt{hp}_{b}",
                           tag=f"qt{hp}_{b}") for b in range(NB)]
                  for hp in range(4)]
            ktr = [pa.tile([128, 512], f16, name=f"ktr{b}", tag=f"ktr{b}")
                   for b in range(NB)]
            kdup = [[pa.tile([128, 512], f16, name=f"kdup{g}_{b}",
                             tag=f"kdup{g}_{b}") for b in range(NB)]
                    for g in range(2)]
            # [V0 | 1 | V1 | 1] — ones column appended per kv head makes PV
            # row 64 the softmax denominator for free
            vtiles = [pa.tile([128, 130], f16, name=f"vt{i}", tag=f"vt{i}")
                      for i in range(NKT)]
            attnT = [[pa.tile([128, 512], f16, name=f"attnT{hp}_{b}",
                              tag=f"attnT{hp}_{b}") for b in range(NB)]
                     for hp in range(4)]

            # ---------------- initial DMAs (spread across queues) ----------
            # first-needed first; 4 queues pull in parallel
            for h in range(4):
                nc.gpsimd.dma_start(out=wk_sb[:, 4 * h:4 * h + 4, :],
                                    in_=wk2[:, 4 * h:4 * h + 4, :])
            nc.sync.dma_start(out=wv_sb, in_=wv2[:, :, :])
            for h in range(4):
                nc.scalar.dma_start(out=wq_sb[:, 4 * h:4 * h + 4, :],
                                    in_=wq2[:, 4 * h:4 * h + 4, :])
            nc.scalar.dma_start(out=cos_sb, in_=cos2[:, :])
            nc.scalar.dma_start(out=sin_sb, in_=sin2[:, :])
            nc.scalar.dma_start(out=wo_sb, in_=wo2[:, :, :])
            make_identity(nc, identity)

            # dummy gpsimd ops up front: force the combined affine_select +
            # partition_broadcast ucode library to load NOW (hidden under the
            # initial DMAs) instead of a ~7us LOAD_LIB stall mid-kernel at
            # the first normalize
            warm = pa.tile([64, 8], f16, name="warm", tag="warm")
            warmf = pa.tile([64, 8], f32, name="warmf", tag="warmf")
            nc.vector.memset(warm, 1.0)
            nc.vector.memset(warmf[0:1, :], 1.0)
            nc.gpsimd.affine_select(
                out=warm, in_=warm, pattern=[[1, 8]],
                compare_op=AOT.is_ge, fill=0.0, base=0,
                channel_multiplier=-1)
            nc.gpsimd.partition_broadcast(out_ap=warmf, in_ap=warmf[0:1, :])

            xt_tiles = {}
            filler = _Filler()

            # ---------------- pipeline units ----------------
            def rope_steps(chunk, b):
                # in-place RoPE on a [128, 512] Q^T/K^T chunk of seq block b
                sl = slice(b * 512, (b + 1) * 512)
                rot = pw.tile([128, 512], f16, name="rot", tag="rot", bufs=3)
                for blk in (0, 64):
                    nc.gpsimd.dma_start(out=rot[blk:blk + 32, :],
                                        in_=chunk[blk + 32:blk + 64, :])
                    nc.gpsimd.dma_start(out=rot[blk + 32:blk + 64, :],
                                        in_=chunk[blk:blk + 32, :])
                yield
                nc.vector.tensor_tensor(out=rot, in0=rot, in1=sin_sb[:, sl],
                                        op=AOT.mult)
                yield
                nc.vector.tensor_tensor(out=chunk, in0=chunk,
                                        in1=cos_sb[:, sl], op=AOT.mult)
                nc.vector.tensor_add(out=chunk, in0=chunk, in1=rot)
                yield

            def emit_xt(b):
                # direct-emitted (not filler) so the prefetch starts early
                t = pw.tile([128, 16, 512], f16, name=f"xt{b}", tag="xt",
                            bufs=3)
                xt_tiles[b] = t
                step = 2 if b == 0 else 4
                eng = nc.sync if b % 2 == 0 else nc.gpsimd
                for ci in range(0, 16, step):
                    eng.dma_start(
                        out=t[:, ci:ci + step, :],
                        in_=xT2[:, ci:ci + step, 512 * b:512 * b + 512])

            def finish_q(b, t, ps):
                dst = qt[t][b]
                nc.vector.tensor_copy(out=dst, in_=ps)
                yield
                yield from rope_steps(dst, b)

            def finish_k(b, ps):
                dst = ktr[b]
                nc.vector.tensor_copy(out=dst, in_=ps)
                yield
                yield from rope_steps(dst, b)
                nc.sync.dma_start(out=kdup[0][b][0:64, :],
                                  in_=dst[0:64, :])
                nc.sync.dma_start(out=kdup[0][b][64:128, :],
                                  in_=dst[0:64, :])
                nc.sync.dma_start(out=kdup[1][b][0:64, :],
                                  in_=dst[64:128, :])
                nc.sync.dma_start(out=kdup[1][b][64:128, :],
                                  in_=dst[64:128, :])
                yield

            def finish_v(b, ps):
                vtr = pw.tile([128, 512], f16, name="vtr", tag="vtr",
                              bufs=2)
                nc.vector.tensor_copy(out=vtr, in_=ps)
                yield
                for ii in range(4):
                    vp = pp.tile([128, 128], f16, name="vt_ps",
                                 tag="qkvps", bufs=2)
                    nc.tensor.transpose(
                        vp, vtr[:, 128 * ii:128 * ii + 128], identity)
                    i = 4 * b + ii
                    nc.vector.tensor_copy(out=vtiles[i][:, 0:64],
                                          in_=vp[:, 0:64])
                    nc.vector.tensor_copy(out=vtiles[i][:, 65:129],
                                          in_=vp[:, 64:128])
                    nc.vector.memset(vtiles[i][:, 64:65], 1.0)
                    nc.vector.memset(vtiles[i][:, 129:130], 1.0)
                    yield

            def proj_unit(b, kind, t=None):
                def gen():
                    xt = xt_tiles[b]
                    ps = pp.tile([128, 512], f32, name="ps", tag="qkvps",
                                 bufs=2)
                    for c in range(16):
                        if kind == "q":
                            lhsT = wq_sb[:, c, 128 * t:128 * t + 128]
                        elif kind == "k":
                            lhsT = wk_sb[:, c, :]
                        else:
                            lhsT = wv_sb[:, c, :]
                        nc.tensor.matmul(ps, lhsT=lhsT, rhs=xt[:, c, :],
                                         start=(c == 0), stop=(c == 15))
                        if c % 2 == 1:
                            yield
                    if kind == "q":
                        yield from finish_q(b, t, ps)
                    elif kind == "k":
                        yield from finish_k(b, ps)
                    else:
                        yield from finish_v(b, ps)
                return gen

            def emit_qkv0_fused():
                # block 0 has nothing to overlap with: loop c OUTER across
                # all 6 accumulators so the matmuls track the xt DMA arrivals
                # instead of serializing unit-by-unit.  Borrows the (idle)
                # stt/pv psum tags for the extra accumulators.
                emit_xt(0)
                xt = xt_tiles[0]
                k_ps = pp.tile([128, 512], f32, name="kps0", tag="stt",
                               bufs=2)
                v_ps = pp.tile([128, 512], f32, name="vps0", tag="stt",
                               bufs=2)
                q_ps = [pp.tile([128, 512], f32, name=f"qps{t}",
                                tag=("qkvps" if t < 2 else "pv"), bufs=2)
                        for t in range(4)]
                for c in range(16):
                    st, sp = (c == 0), (c == 15)
                    nc.tensor.matmul(k_ps, lhsT=wk_sb[:, c, :],
                                     rhs=xt[:, c, :], start=st, stop=sp)
                    for t in range(4):
                        nc.tensor.matmul(
                            q_ps[t], lhsT=wq_sb[:, c, 128 * t:128 * t + 128],
                            rhs=xt[:, c, :], start=st, stop=sp)
                    nc.tensor.matmul(v_ps, lhsT=wv_sb[:, c, :],
                                     rhs=xt[:, c, :], start=st, stop=sp)
                for _ in finish_k(0, k_ps):
                    pass
                for _ in finish_q(0, 0, q_ps[0]):
                    pass
                for _ in finish_v(0, v_ps):
                    pass
                for t in (1, 2, 3):
                    for _ in finish_q(0, t, q_ps[t]):
                        pass
                for lab in ("xt@0", "k@0", "q0@0", "v@0",
                            "q1@0", "q2@0", "q3@0"):
                    filler.done.add(lab)

            def oproj_unit(b):
                def gen():
                    for s_ in range(4 * b, 4 * b + 4):
                        so = (s_ - 4 * b) * 128
                        ostage = pw.tile([128, DIM], f16, name="ostage",
                                         tag="ostage", bufs=2)
                        for dm in range(4):
                            ops = pp.tile([128, 512], f32, name="ops",
                                          tag="qkvps", bufs=2)
                            for c in range(4):
                                nc.tensor.matmul(
                                    ops, lhsT=attnT[c][b][:, so:so + 128],
                                    rhs=wo_sb[:, c, dm * 512:(dm + 1) * 512],
                                    start=(c == 0), stop=(c == 3))
                                if c % 2 == 1:
                                    yield
                            nc.vector.tensor_copy(
                                out=ostage[:, dm * 512:(dm + 1) * 512],
                                in_=ops)
                            yield
                        nc.sync.dma_start(
                            out=outp[128 * s_:128 * s_ + 128, :], in_=ostage)
                        yield
                return gen

            def emit_qkv(b):
                emit_xt(b)
                filler.add(f"k@{b}", proj_unit(b, "k"))
                filler.add(f"q0@{b}", proj_unit(b, "q", 0))
                filler.add(f"v@{b}", proj_unit(b, "v"))
                filler.add(f"q1@{b}", proj_unit(b, "q", 1))
                filler.add(f"q2@{b}", proj_unit(b, "q", 2))
                filler.add(f"q3@{b}", proj_unit(b, "q", 3))

            # ---------------- attention ----------------
            # PV(kt) emission is deferred one iteration — emitted only after
            # the NEXT scores+exp, so the in-order tensor queue never
            # head-of-line blocks on exp(kt).  The pending PV carries across
            # hp and block boundaries; its exp is ~a full period old by
            # flush time.  normalize rides along when the pending PV closes
            # the accumulation (sp flag).
            pvstate = {"pend": deque()}

            def normalize(pv_, hp_, b_, par):
                den = pw.tile([1, 512], f32, name="den", tag="den", bufs=4)
                nc.vector.tensor_copy(out=den, in_=pv_[64:65, :])
                rec = pw.tile([1, 512], f32, name="rec", tag="rec", bufs=4)
                nc.vector.reciprocal_approx_fast(out=rec, in_=den)
                rbc = pw.tile([64, 512], f32, name="rbc", tag="rbc", bufs=4)
                nc.gpsimd.partition_broadcast(out_ap=rbc, in_ap=rec)
                nc.vector.tensor_tensor(
                    out=attnT[hp_][b_][64 * par:64 * par + 64, :],
                    in0=pv_[0:64, :], in1=rbc, op=AOT.mult)

            def flush_one():
                if not pvstate["pend"]:
                    return
                pv_, hp_, b_, g_, pt_, lo_, st_, sp_, kt_ = \
                    pvstate["pend"].popleft()
                for h in (0, 1):
                    nc.tensor.matmul(
                        pv_[h][:, lo_:512],
                        lhsT=vtiles[kt_][:, 65 * g_:65 * g_ + 65],
                        rhs=pt_[:, h, lo_:512], start=st_, stop=sp_)
                if sp_:
                    for par in (0, 1):
                        normalize(pv_[par], hp_, b_, par)

            def flush_pv():
                while pvstate["pend"]:
                    flush_one()

            def attention(b):
                nkt = 4 * b + 4 if causal else NKT
                filler.run_until(f"q0@{b}")
                filler.run_until(f"v@{b}")
                for hp in range(4):
                    filler.run_until(f"q{hp}@{b}")
                    # flush the previous pair's pending PV before its psum
                    # slots rotate (bufs=2, two allocs per pair)
                    flush_pv()
                    filler.step(1)
                    g = hp // 2  # local kv head shared by the pair
                    pv2 = [pp.tile([65, 512], f32, name=f"pv{par}", tag="pv",
                                   bufs=2) for par in range(2)]
                    for kt in range(nkt):
                        i = kt - 4 * b  # diagonal offset (>=0 on/after diag)
                        lo = 128 * i if (causal and i >= 0) else 0
                        stt = pp.tile([128, 2, 512], f32, name="stt",
                                      tag="stt", bufs=2)
                        lk = kdup[g][kt // 4]
                        ck = slice((kt % 4) * 128, (kt % 4 + 1) * 128)
                        for h in (0, 1):
                            nc.tensor.matmul(
                                stt[:, h, lo:512],
                                lhsT=lk[64 * h:64 * h + 64, ck],
                                rhs=qt[hp][b][64 * h:64 * h + 64, lo:512],
                                start=True, stop=True,
                                tile_position=(64 * h, 0))
                        pt = pw.tile([128, 2, 512], f16, name="pt", tag="pt",
                                     bufs=8)
                        nc.scalar.activation(out=pt[:, :, lo:512],
                                             in_=stt[:, :, lo:512],
                                             func=EXP, scale=SCALE)
                        if causal and i >= 0:
                            # zero the strictly-above-diagonal triangle
                            nc.gpsimd.affine_select(
                                out=pt[:, :, lo:lo + 128],
                                in_=pt[:, :, lo:lo + 128],
                                pattern=[[0, 2], [1, 128]],
                                compare_op=AOT.is_ge,
                                fill=0.0, base=0, channel_multiplier=-1)
                        if kt == 0 and hp < 3:
                            # pull the next head-pair's Q unit forward so its
                            # RoPE latency hides under this pair's kt loop
                            filler.run_until(f"q{hp + 1}@{b}")
                        if kt == 2 and causal and b + 1 < NB:
                            # pull the next block's K/Q0/V units deep into
                            # this block so their RoPE/kdup/transpose chains
                            # finish long before attention(b+1) starts
                            if hp == 0:
                                filler.run_until(f"k@{b + 1}")
                            elif hp == 1:
                                filler.run_until(f"q0@{b + 1}")
                            elif hp == 2:
                                filler.run_until(f"v@{b + 1}")
                        # PV(kt) flushes two iterations late: its exp is two
                        # periods old by then, so it never blocks the queue
                        if len(pvstate["pend"]) >= 4:
                            flush_one()
                        filler.step(2 if hp == 3 else 1)
                        pvstate["pend"].append((pv2, hp, b, g, pt, lo,
                                                kt == 0, kt == nkt - 1, kt))

            # ---------------- main pipeline ----------------
            if causal:
                emit_qkv0_fused()
                for b in range(NB):
                    if b + 1 < NB:
                        emit_qkv(b + 1)
                    # oproj filler delayed two blocks: attention(3) is
                    # scalar(exp)-bound with no QKV left, so it gets both
                    # oproj(1) and oproj(2) to fill the tensor slack
                    if b == 2:
                        filler.add("oproj@0", oproj_unit(0))
                    elif b == 3:
                        filler.add("oproj@1", oproj_unit(1))
                        filler.add("oproj@2", oproj_unit(2))
                    attention(b)
                flush_pv()
                filler.run_all()
                for _ in oproj_unit(NB - 1)():
                    pass
            else:
                for b in range(NB):
                    emit_qkv(b)
                    filler.run_all()
                for b in range(NB):
                    if b - 1 >= 0:
                        filler.add(f"oproj@{b - 1}", oproj_unit(b - 1))
                    attention(b)
                flush_pv()
                filler.run_all()
                for _ in oproj_unit(NB - 1)():
                    pass

    nc.compile()
    return nc


def _get_program(causal: bool):
    key = ("v2", causal)
    if key not in _PROGRAM_CACHE:
        _PROGRAM_CACHE[key] = _build_program(causal)
    return _PROGRAM_CACHE[key]


def _check_causal(mask: np.ndarray) -> bool:
    m = mask.reshape(SEQ, SEQ)
    idx = np.array([0, 1, 7, 100, 1000, 2047])
    sub = m[np.ix_(idx, idx)]
    expect_zero = idx[:, None] >= idx[None, :]
    if not np.all(sub[expect_zero] == 0.0):
        return False
    if not np.all(sub[~expect_zero] < -1e30):
        return False
    return True


def _tile_pm(a: np.ndarray, nchunk: int) -> np.ndarray:
    """[nchunk*128, n] -> partition-major [128, nchunk, n] fp16."""
    n = a.shape[1]
    return np.ascontiguousarray(
        a.reshape(nchunk, 128, n).transpose(1, 0, 2).astype(np.float16))


def kernel(x, Wq, Wk, Wv, Wo, cos, sin, attention_mask):
    from concourse.bass_utils import run_bass_kernel_spmd

    x = np.asarray(x, dtype=np.float32)
    Wq = np.asarray(Wq, dtype=np.float32)
    Wk = np.asarray(Wk, dtype=np.float32)
    Wv = np.asarray(Wv, dtype=np.float32)
    Wo = np.asarray(Wo, dtype=np.float32)
    cos = np.asarray(cos, dtype=np.float32)
    sin = np.asarray(sin, dtype=np.float32)
    mask = np.asarray(attention_mask, dtype=np.float32)

    causal = _check_causal(mask)
    if not causal:
        assert np.all(mask == 0.0), (
            "kernel only supports the causal or all-zero attention masks")

    # host-preprocessed RoPE tables: transposed, duplicated to 128 partitions,
    # sign folded into sin for the rotate_half shift
    cosT = np.ascontiguousarray(cos.T)  # (64, SEQ)
    sinT = sin.T
    sin_signed = np.concatenate([-sinT[:32], sinT[32:]], axis=0)
    cos2 = np.ascontiguousarray(np.tile(cosT, (2, 1))).astype(np.float16)
    sin2 = np.ascontiguousarray(np.tile(sin_signed, (2, 1))).astype(np.float16)

    nc = _get_program(causal)

    in_maps = []
    for core in range(N_CORES):
        b, g4 = core // 4, core % 4
        in_maps.append({
            "xT2": _tile_pm(np.ascontiguousarray(x[b].T), 16),
            "wq2": _tile_pm(Wq[:, g4 * DQ:(g4 + 1) * DQ], 16),
            "wk2": _tile_pm(Wk[:, g4 * DKV:(g4 + 1) * DKV], 16),
            "wv2": _tile_pm(Wv[:, g4 * DKV:(g4 + 1) * DKV], 16),
            "wo2": _tile_pm(Wo[g4 * DQ:(g4 + 1) * DQ, :], 4),
            "cos2": cos2,
            "sin2": sin2,
        })

    trace = bool(int(os.environ.get("KERNEL_TRACE", "0")))
    res = run_bass_kernel_spmd(nc, in_maps, list(range(N_CORES)), trace=trace)
    if trace:
        kernel.last_exec_time_ns = res.exec_time_ns
        kernel.last_profile = res.profile_json

    outs = [res.results[i]["out"].astype(np.float32) for i in range(N_CORES)]
    y0 = outs[0] + outs[1] + outs[2] + outs[3]
    y1 = outs[4] + outs[5] + outs[6] + outs[7]
    return np.stack([y0, y1]).astype(np.float32)
